# revision 10
# baseline (speedup 1.0000x reference)
"""Trainium2 Bass kernel for nn_AttentionModel (sparse banded attention).

Math (per batch element, data-parallel over 8 cores):
  qs    = q @ W_score.T
  score = qs @ k.T                      # only the 129-wide causal band matters
  w     = banded_softmax(score)         # full-row max cancels mathematically
  c     = w @ k
  enh   = tanh(concat([c, q]) @ W_enh.T + b_enh)
  out   = sigmoid(enh @ W_mask.T + b_mask)

Implementation (v3, transposed-score structure, all-bf16 operands):
  - T=2000 padded to 2048 on both query and key axes (16 blocks of 128).
    Key block m holds scores for query tiles j=m (diagonal relation, keep
    s'<=t') and j=m+1 (previous-block relation, keep s'>=t').
  - Scores are computed TRANSPOSED per key block: psum[s',t'2tiles] =
    kT_blk^T @ qsT window.  Band masking is a DVE tensor_tensor add of a
    single constant [diag|prev] mask; exp runs on ACT writing bf16 w~T tiles
    that are already in the layout PV wants (no w transposes).
  - PV appends a ones column to k: c~[t',258] = w~T.T @ [kN|1], so column 256
    is the softmax denominator for free.  Normalization is one reciprocal +
    one per-partition tensor_scalar multiply per tile.  c is then transposed
    (PE, bf16, 1 cyc/row) into feature-major cT for the enhancement matmul.
  - Final stage is computed TRANSPOSED (outT[o,t']) so b_mask rides the ACT
    per-partition bias port, sigmoid(x)=0.5*tanh(0.5x)+0.5 stays in the
    exp_and_others table set, and the output DMA gets wide bf16 rows.
    The host un-transposes and upcasts.  The last 512 output columns are
    processed as two 256-wide groups so the drain tail is short.
  - All matmul operands are bf16 (1 cyc/row on the PE, measured end-to-end
    rel err 1.1e-2 vs the 2e-2 gate); PSUM accumulation is fp32 throughout.
  - Each dma_start binds a single ~18GB/s DMA engine, so loads are split
    into <=65KB pieces issued from five queues (sync/vector/scalar/gpsimd/
    tensor) in consumption-priority order; bulk pieces drip between early
    compute ops so the first P0 matmul starts as soon as possible.
  - PE stream is software-pipelined: scores run 2 blocks ahead of PV,
    3 ahead of the c transposes, with P2/P3 groups interleaved one step
    apart, so the DVE-mask -> ACT-exp chain never stalls the tensor engine.
"""

import sys
import types

import numpy as np
import ml_dtypes
from contextlib import ExitStack

import concourse.bass as bass
import concourse.bacc as bacc
import concourse.tile as tile
from concourse import mybir
from concourse.bass_utils import run_bass_kernel_spmd


def _ensure_axon_hooks():
    try:
        from antenv import axon_hooks  # noqa: F401
        return
    except ImportError:
        pass
    try:
        from trn_agent_boot.trn_boot import _ntff_profile_via_ctypes
        hook = _ntff_profile_via_ctypes("/opt/axon/libaxon_pjrt.so")
    except Exception:
        hook = None
    m = types.ModuleType("antenv.axon_hooks")
    m.get_axon_ntff_profile_hook = lambda: hook
    m.set_axon_ntff_profile_hook = lambda h: None
    sys.modules["antenv.axon_hooks"] = m


_ensure_axon_hooks()

F32 = mybir.dt.float32
BF16 = mybir.dt.bfloat16
AF = mybir.ActivationFunctionType
ALU = mybir.AluOpType

B, T, H, F_OUT = 8, 2000, 256, 257
TQ = 2048          # padded query/key length (16 tiles of 128)
NT = 16            # tiles/blocks of 128
KW = 258           # kN row width: 256 features + ones col + pad col
NEG = -30000.0
N_CORES = 8

_CACHE = {}


def build_nc():
    nc = bacc.Bacc("TRN2", target_bir_lowering=False, debug=False,
                   num_devices=N_CORES)

    qT = nc.declare_dram_parameter("qT", [H, TQ], BF16, isOutput=False)
    kT = nc.declare_dram_parameter("kT", [H, TQ], BF16, isOutput=False)
    kN = nc.declare_dram_parameter("kN", [128, NT * KW], BF16, isOutput=False)
    WsT = nc.declare_dram_parameter("WsT", [H, H], BF16, isOutput=False)
    WeTq = nc.declare_dram_parameter("WeTq", [H, H], BF16, isOutput=False)
    blobF = nc.declare_dram_parameter("blobF", [128, 261], F32, isOutput=False)
    blobB = nc.declare_dram_parameter("blobB", [128, 1664], BF16,
                                      isOutput=False)
    out = nc.declare_dram_parameter("out", [KW, TQ], BF16, isOutput=True)

    with tile.TileContext(nc) as tc, ExitStack() as ctx:
        const = ctx.enter_context(tc.tile_pool(name="const", bufs=1))
        io = ctx.enter_context(tc.tile_pool(name="io", bufs=1))
        wk = ctx.enter_context(tc.tile_pool(name="wk", bufs=1))
        stat = ctx.enter_context(tc.tile_pool(name="stat", bufs=1))
        pmm = ctx.enter_context(tc.tile_pool(name="pmm", bufs=2, space="PSUM"))
        psc = ctx.enter_context(tc.tile_pool(name="psc", bufs=2, space="PSUM"))
        ppv = ctx.enter_context(tc.tile_pool(name="ppv", bufs=2, space="PSUM"))
        ptr = ctx.enter_context(tc.tile_pool(name="ptr", bufs=2, space="PSUM"))

        # ---- persistent SBUF ----
        wst = [const.tile([128, H], BF16, tag=f"wst{c}", name=f"wst{c}")
               for c in range(2)]
        weq = [const.tile([128, H], BF16, tag=f"weq{c}", name=f"weq{c}")
               for c in range(2)]
        blobF_t = const.tile([128, 261], F32, tag="blobF", name="blobF_t")
        blobB_t = const.tile([128, 1664], BF16, tag="blobB", name="blobB_t")

        qT_t = [io.tile([128, TQ], BF16, tag=f"qT{c}", name=f"qT{c}")
                for c in range(2)]
        kT_t = [io.tile([128, TQ], BF16, tag=f"kT{c}", name=f"kT{c}")
                for c in range(2)]
        kN_t = io.tile([128, NT * KW], BF16, tag="kN", name="kN_t")
        # 128 pad cols so score block 15 streams a full 256-wide window
        qsT_t = [io.tile([128, TQ + 128], BF16, tag=f"qsT{c}", name=f"qsT{c}")
                 for c in range(2)]
        cT_t = [io.tile([128, TQ], BF16, tag=f"cT{c}", name=f"cT{c}")
                for c in range(2)]
        enhT_t = [io.tile([128, TQ], BF16, tag=f"enhT{c}", name=f"enhT{c}")
                  for c in range(2)]
        oT_sb = [io.tile([128, TQ], BF16, tag=f"oT{c}", name=f"oT{c}")
                 for c in range(2)]
        oT_row = io.tile([1, TQ], BF16, tag="oTr", name="oT_row")

        # const views
        beS = [blobF_t[:, 256 + f:257 + f] for f in range(2)]
        bmS = [blobF_t[:, 258 + ci:259 + ci] for ci in range(3)]
        identB = blobB_t[:, 0:128]
        web = [blobB_t[:, 128 + d * 256: 128 + (d + 1) * 256] for d in range(2)]
        wmp = [blobB_t[:, 640 + f * 384: 640 + (f + 1) * 384] for f in range(2)]
        mask01 = blobB_t[:, 1408:1664]    # [diag s'<=t' | prev s'>=t'] as 1/0

        # ---- DMA load pieces, priority-ordered per issue queue ----
        # Only sync/scalar(ACT)/gpsimd queues can issue DMAs (~0.6us per
        # issue, serial per queue).  Each dma_start binds one DMA engine
        # (~18GB/s), so early pieces are 64KB halves ordered by consumption
        # time; late pieces are bigger.  All issued up-front.
        def hp(dst_tile, src, r0, c0, c1):
            return [(dst_tile[0:64, c0:c1], src[r0:r0 + 64, c0:c1]),
                    (dst_tile[64:128, c0:c1], src[r0 + 64:r0 + 128, c0:c1])]

        sync_loads = (
            [(wst[0][:, 0:128], WsT[0:128, 0:128]),
             (qT_t[0][0:64, 0:512], qT[0:64, 0:512]),
             (kT_t[0][0:64, 0:512], kT[0:64, 0:512])]
            + hp(qT_t[0], qT, 0, 512, 1024)
            + hp(kT_t[0], kT, 0, 512, 1024)
            + hp(qT_t[0], qT, 0, 1024, 1536)
            + hp(kT_t[1], kT, 128, 512, 1024)
            + hp(qT_t[0], qT, 0, 1536, 2048))
        for d, s in sync_loads:
            nc.sync.dma_start(d, s)

        sca_head = [
            (wst[1][:, 0:128], WsT[128:256, 0:128]),
            (qT_t[1][0:64, 0:512], qT[128:192, 0:512]),
            (kT_t[1][0:64, 0:512], kT[128:192, 0:512]),
            (wst[1][:, 128:256], WsT[128:256, 128:256]),
            (qT_t[1][64:128, 0:512], qT[192:256, 0:512]),
            (kT_t[1][64:128, 0:512], kT[192:256, 0:512]),
        ]
        for d, s in sca_head:
            nc.scalar.dma_start(d, s)

        gps_loads = (
            [(wst[0][:, 128:256], WsT[0:128, 128:256]),
             (qT_t[0][64:128, 0:512], qT[64:128, 0:512]),
             (kT_t[0][64:128, 0:512], kT[64:128, 0:512]),
             (blobB_t[:, 1408:1664], blobB[:, 1408:1664])]   # mask01
            + hp(qT_t[0], qT, 0, 0, 512)[0:0]  # placeholder, keeps diff sane
            + [(blobB_t[:, 0:128], blobB[:, 0:128])]
            + hp(qT_t[1], qT, 128, 512, 1024)
            + [(kN_t[:, 0:2 * KW], kN[:, 0:2 * KW])]
            + hp(qT_t[1], qT, 128, 1024, 1536)
            + [(kN_t[:, 2 * KW:4 * KW], kN[:, 2 * KW:4 * KW]),
               (blobB_t[:, 640:1024], blobB[:, 640:1024])]      # wmp0
            + hp(qT_t[1], qT, 128, 1536, 2048)
            + [(kN_t[:, 4 * KW:6 * KW], kN[:, 4 * KW:6 * KW]),
               (blobB_t[:, 128:640], blobB[:, 128:640]),        # web
               (blobB_t[:, 1024:1408], blobB[:, 1024:1408]),    # wmp1
               (kN_t[:, 6 * KW:8 * KW], kN[:, 6 * KW:8 * KW]),
               (kT_t[0][:, 1024:1536], kT[0:128, 1024:1536]),
               (kT_t[1][:, 1024:1536], kT[128:256, 1024:1536]),
               (kN_t[:, 8 * KW:10 * KW], kN[:, 8 * KW:10 * KW]),
               (weq[0][:], WeTq[0:128, :]),
               (weq[1][:], WeTq[128:256, :]),
               (kN_t[:, 10 * KW:12 * KW], kN[:, 10 * KW:12 * KW]),
               (kT_t[0][:, 1536:2048], kT[0:128, 1536:2048]),
               (kT_t[1][:, 1536:2048], kT[128:256, 1536:2048]),
               (kN_t[:, 12 * KW:14 * KW], kN[:, 12 * KW:14 * KW]),
               (kN_t[:, 14 * KW:16 * KW], kN[:, 14 * KW:16 * KW]),
               (blobF_t[0:64, :], blobF[0:64, :]),
               (blobF_t[64:128, :], blobF[64:128, :])])
        for d, s in gps_loads:
            nc.gpsimd.dma_start(d, s)

        # ---- stage emitters ----
        def emit_p0(nb):
            # qsT[g, t'] = (q @ W_score.T).T : stationary wst, moving qT.
            # h-chunks interleaved across the two g psums so the first
            # matmuls only need qT_t[0]'s first piece.
            ps = [pmm.tile([128, 512], F32, tag="mm", name=f"p0_{c}")
                  for c in range(2)]
            for h in range(2):
                for c in range(2):
                    nc.tensor.matmul(
                        ps[c][:],
                        wst[h][:, c * 128:(c + 1) * 128],
                        qT_t[h][:, nb * 512:(nb + 1) * 512],
                        start=(h == 0), stop=(h == 1))
            for c in range(2):
                nc.vector.tensor_copy(qsT_t[c][:, nb * 512:(nb + 1) * 512],
                                      ps[c][:])

        def emit_sc(m):
            # scoreT[s' of block m, t' of tiles m,m+1] + mask -> exp -> w~T
            # (block 15's upper half reads qsT pad cols: garbage, never used)
            ps = psc.tile([128, 256], F32, tag="sc", name="ps")
            for g in range(2):
                nc.tensor.matmul(
                    ps[:],
                    kT_t[g][:, m * 128:(m + 1) * 128],
                    qsT_t[g][:, m * 128: m * 128 + 256],
                    start=(g == 0), stop=(g == 1))
            wt = wk.tile([128, 256], BF16, tag="wt", bufs=7, name="wt")
            nc.scalar.activation(wt[:], ps[:], AF.Exp)
            nc.vector.tensor_mul(wt[:], wt[:], mask01)
            return wt

        wT = [None] * NT

        def emit_pv(j):
            # c~[t', 258] = sum_blocks w~T.T @ [kN | 1]
            pc = ppv.tile([128, KW], F32, tag="pv", name="pc")
            if j == 0:
                nc.tensor.matmul(pc[:], wT[0][:, 0:128],
                                 kN_t[:, 0:KW], start=True, stop=True)
            else:
                nc.tensor.matmul(pc[:], wT[j - 1][:, 128:256],
                                 kN_t[:, (j - 1) * KW: j * KW],
                                 start=True, stop=False)
                nc.tensor.matmul(pc[:], wT[j][:, 0:128],
                                 kN_t[:, j * KW: (j + 1) * KW],
                                 start=False, stop=True)
            rec = stat.tile([128, 1], F32, tag="rec", bufs=4, name="rec")
            nc.vector.reciprocal(rec[:], pc[:, 256:257])
            cb = wk.tile([128, 256], BF16, tag="cb", bufs=4, name="cb")
            nc.vector.tensor_scalar_mul(cb[:], pc[:, 0:256], rec[:])
            return cb

        cB = [None] * NT

        def emit_tr(j):
            # cT[h, t'] via PE transpose (bf16); psum->sbuf on DVE and ACT
            for h in range(2):
                pt = ptr.tile([128, 128], BF16, tag="tr", name="pt")
                nc.tensor.transpose(pt[:], cB[j][:, h * 128:(h + 1) * 128],
                                    identB)
                if h == 0:
                    nc.vector.tensor_copy(cT_t[h][:, j * 128:(j + 1) * 128],
                                          pt[:])
                else:
                    nc.scalar.copy(cT_t[h][:, j * 128:(j + 1) * 128], pt[:])

        # P2/P3 groups: (col0, width) pairs; last 512 split into 2x256
        GROUPS = [(0, 512), (512, 512), (1024, 512), (1536, 256), (1792, 256)]

        def emit_p2(gi):
            c0, w = GROUPS[gi]
            for f in range(2):
                pe_ = pmm.tile([128, 512], F32, tag="mm", name="pe_")
                nc.tensor.matmul(pe_[:, 0:w], web[0][:, f * 128:(f + 1) * 128],
                                 cT_t[0][:, c0:c0 + w],
                                 start=True, stop=False)
                nc.tensor.matmul(pe_[:, 0:w], web[1][:, f * 128:(f + 1) * 128],
                                 cT_t[1][:, c0:c0 + w],
                                 start=False, stop=False)
                nc.tensor.matmul(pe_[:, 0:w], weq[0][:, f * 128:(f + 1) * 128],
                                 qT_t[0][:, c0:c0 + w],
                                 start=False, stop=False)
                nc.tensor.matmul(pe_[:, 0:w], weq[1][:, f * 128:(f + 1) * 128],
                                 qT_t[1][:, c0:c0 + w],
                                 start=False, stop=True)
                nc.scalar.activation(enhT_t[f][:, c0:c0 + w],
                                     pe_[:, 0:w], AF.Tanh, bias=beS[f])

        def emit_p3(gi):
            c0, w = GROUPS[gi]
            for ci in range(3):
                p3 = pmm.tile([128, 512], F32, tag="mm", name="p3")
                for f in range(2):
                    nc.tensor.matmul(
                        p3[:, 0:w], wmp[f][:, ci * 128:(ci + 1) * 128],
                        enhT_t[f][:, c0:c0 + w],
                        start=(f == 0), stop=(f == 1))
                if ci < 2:
                    os = wk.tile([128, 512], BF16, tag="os", bufs=2, name="os")
                    nc.scalar.activation(os[:, 0:w], p3[:, 0:w], AF.Tanh,
                                         scale=0.5, bias=bmS[ci])
                    nc.gpsimd.tensor_scalar(
                        oT_sb[ci][:, c0:c0 + w], os[:, 0:w],
                        0.5, 0.5, op0=ALU.mult, op1=ALU.add)
                    if gi < 3:
                        nc.sync.dma_start(out[ci * 128:(ci + 1) * 128,
                                              c0:c0 + w],
                                          oT_sb[ci][:, c0:c0 + w])
                    else:
                        # tail: small pieces across queues for low latency
                        r0 = ci * 128
                        nc.sync.dma_start(out[r0:r0 + 32, c0:c0 + w],
                                          oT_sb[ci][0:32, c0:c0 + w])
                        nc.scalar.dma_start(out[r0 + 32:r0 + 64, c0:c0 + w],
                                            oT_sb[ci][32:64, c0:c0 + w])
                        nc.gpsimd.dma_start(out[r0 + 64:r0 + 96, c0:c0 + w],
                                            oT_sb[ci][64:96, c0:c0 + w])
                        nc.sync.dma_start(out[r0 + 96:r0 + 128, c0:c0 + w],
                                          oT_sb[ci][96:128, c0:c0 + w])
                else:
                    os1 = wk.tile([1, 512], BF16, tag="os1", bufs=2,
                                  name="os1")
                    nc.scalar.activation(os1[:, 0:w], p3[0:1, 0:w], AF.Tanh,
                                         scale=0.5, bias=bmS[2][0:1, :])
                    nc.gpsimd.tensor_scalar(
                        oT_row[0:1, c0:c0 + w], os1[:, 0:w],
                        0.5, 0.5, op0=ALU.mult, op1=ALU.add)
                    nc.scalar.dma_start(out[256:257, c0:c0 + w],
                                        oT_row[0:1, c0:c0 + w])

        # ---- software-pipelined emission ----
        # pv lags scores by 4 steps, transposes by 5; p2 for group gi fires
        # at step P2STEP[gi], p3 one step later.
        LPV, LTR = 4, 5
        P2STEP = {9: 0, 13: 1, 17: 2, 19: 3, 20: 4}
        P3STEP = {10: 0, 14: 1, 18: 2, 20: 3, 21: 4}

        def emit_lagged(step):
            jpv = step - LPV
            if 0 <= jpv < NT:
                cB[jpv] = emit_pv(jpv)
            jtr = step - LTR
            if 0 <= jtr < NT:
                emit_tr(jtr)
            if step in P2STEP:
                emit_p2(P2STEP[step])
            if step in P3STEP:
                emit_p3(P3STEP[step])

        m_next = 0
        for nb in range(4):
            emit_p0(nb)
            hi = 4 * nb + 2 if nb < 3 else NT - 1
            while m_next <= hi:
                wT[m_next] = emit_sc(m_next)
                emit_lagged(m_next)
                m_next += 1
        for step in range(NT, NT + 6):
            emit_lagged(step)

    return nc


def make_in_maps(k, q, W_score, W_enh, b_enh, W_mask, b_mask):
    k = np.asarray(k, np.float32)
    q = np.asarray(q, np.float32)
    W_score = np.asarray(W_score, np.float32)
    W_enh = np.asarray(W_enh, np.float32)
    b_enh = np.asarray(b_enh, np.float32)
    W_mask = np.asarray(W_mask, np.float32)
    b_mask = np.asarray(b_mask, np.float32)

    bf = ml_dtypes.bfloat16
    WsT = np.ascontiguousarray(W_score.T).astype(bf)       # [h, g]
    WeT = W_enh.T                                          # [d, f] (512, 256)
    WeTq = np.ascontiguousarray(WeT[H:2 * H]).astype(bf)   # q-feature half

    sI = np.arange(128, dtype=np.int32)[:, None]
    tI = np.arange(128, dtype=np.int32)[None, :]
    diag = np.where(sI <= tI, 0.0, NEG).astype(np.float32)
    prev = np.where(sI >= tI, 0.0, NEG).astype(np.float32)
    blobF = np.zeros((128, 261), np.float32)
    blobF[:, 0:128] = diag
    blobF[:, 128:256] = prev
    blobF[:, 256:258] = b_enh.reshape(2, 128).T
    bmh = np.zeros(384, np.float32)
    bmh[:F_OUT] = 0.5 * b_mask
    blobF[:, 258:261] = bmh.reshape(3, 128).T

    blobB = np.zeros((128, 1664), np.float32)
    blobB[:, 0:128] = np.eye(128, dtype=np.float32)
    blobB[:, 128:384] = WeT[0:128]
    blobB[:, 384:640] = WeT[128:256]
    WmP = np.zeros((H, 384), np.float32)
    WmP[:, :F_OUT] = W_mask.T
    blobB[:, 640:1024] = WmP[0:128]
    blobB[:, 1024:1408] = WmP[128:256]
    blobB[:, 1408:1536] = (sI <= tI).astype(np.float32)   # diag keep
    blobB[:, 1536:1664] = (sI >= tI).astype(np.float32)   # prev keep
    blobB = blobB.astype(bf)

    in_maps = []
    for b in range(N_CORES):
        kb = np.zeros((TQ, H), np.float32)
        kb[:T] = k[b]
        qb = np.zeros((TQ, H), np.float32)
        qb[:T] = q[b]
        kNb = np.zeros((TQ, KW), np.float32)
        kNb[:, 0:H] = kb
        kNb[:, 256] = 1.0
        # pre-rearrange into the SBUF layout [p, block*KW + h]
        kNr = np.ascontiguousarray(
            kNb.reshape(NT, 128, KW).transpose(1, 0, 2).reshape(128, NT * KW))
        in_maps.append({
            "qT": np.ascontiguousarray(qb.T).astype(bf),
            "kT": np.ascontiguousarray(kb.T).astype(bf),
            "kN": kNr.astype(bf),
            "WsT": WsT, "WeTq": WeTq,
            "blobF": blobF, "blobB": blobB,
        })
    return in_maps


def assemble_output(results):
    outs = []
    for r in results:
        o = np.asarray(r["out"]).astype(np.float32)         # [258, 2048]
        outs.append(np.ascontiguousarray(o[:F_OUT, :T].T))  # [2000, 257]
    return np.stack(outs, 0)


def get_nc():
    if "nc" not in _CACHE:
        nc = build_nc()
        nc.finalize()
        _CACHE["nc"] = nc
    return _CACHE["nc"]


def kernel(k, q, W_score, W_enh, b_enh, W_mask, b_mask):
    in_maps = make_in_maps(k, q, W_score, W_enh, b_enh, W_mask, b_mask)
    res = run_bass_kernel_spmd(get_nc(), in_maps, list(range(N_CORES)))
    return assemble_output(res.results)
